# revision 40
# baseline (speedup 1.0000x reference)
"""Trainium2 Bass kernel for nn_MultiHeadAttention_76587856823057.

Sharding: (batch, query-half) -> 8 cores, zero collectives.
Per core: b fixed, queries TQ=1024 (half of T), all H=16 heads, all TK=2048 keys.

Math notes vs reference:
 - softmax is shift-invariant; the reference's *global* max subtract cancels in
   the normalization (the +1e-15 in the denominator is ~1e-12 relative), and
   scores are bounded (|s| < ~30) so exp() cannot overflow in fp32. We skip the
   max pass entirely.
 - exp(s*m)*m == exp(s)*m for m in {0,1}: one mask multiply only.
 - row sums come free from the PV matmul via a ones-column per head (M=65).
 - Q projection runs in float32r (fp32 with 11-bit-mantissa inputs, fp32
   accumulate, full PE speed); K/V projections, scores and probabilities in
   bf16 (fast weight loads, no fp32 two-pass weight path).
 - QK contracts K=128 against parity-zero-padded qhT_z so every matmul stays
   in plain 128x128 PE mode (no tile-mode switching / drains).
 - K-projection is interleaved into the (ACT-bound) attention phase, one
   feature-tile block per pair, directly producing each pair's khp SBUF tile.
 - biases fold into the matmuls via an appended ones-row (K=1025).

Self-contained: hardcodes all shapes; no sibling imports.
"""

import os
import numpy as np

import concourse.bass as bass
from concourse import bacc
import concourse.mybir as mybir
from concourse.tile import TileContext
from concourse.bass_utils import run_bass_kernel_spmd
from concourse.masks import make_identity

F32 = mybir.dt.float32
F32R = mybir.dt.float32r
BF16 = mybir.dt.bfloat16
AF = mybir.ActivationFunctionType

B, T, D, H, DK = 4, 2048, 1024, 16, 64
TQ = T // 2          # queries per core
TK = T               # keys per core
NCORES = 8
NPAIR = H // 2       # 8 head pairs
NFT = D // 128       # 8 feature tiles
NKT = TK // 128      # 16 key tiles
VEXT = H * (DK + 1)  # 1040: per-head [64 v-cols + ones col]

_LAST_RESULTS = {}


def _round_f32r(x: np.ndarray) -> np.ndarray:
    """Round fp32 to the PE's fp32r input format (11-bit mantissa)."""
    bits = np.ascontiguousarray(x, dtype=np.float32).view(np.uint32)
    out = ((bits.astype(np.uint64) + 0x800) & 0xFFFFF000).astype(np.uint32)
    return out.view(np.float32)


def build_program(nc: bass.Bass, trivial_affine: bool = False):
    # ---- per-core DRAM I/O ----
    qT = nc.dram_tensor("qT", [D + 1, TQ], F32R, kind="ExternalInput").ap()
    kT = nc.dram_tensor("kT", [D + 1, TK], BF16, kind="ExternalInput").ap()
    vT = nc.dram_tensor("vT", [D + 1, TK], BF16, kind="ExternalInput").ap()
    wq = nc.dram_tensor("wq", [D + 1, D], F32R, kind="ExternalInput").ap()
    wk = nc.dram_tensor("wk", [D + 1, D], BF16, kind="ExternalInput").ap()
    wv = nc.dram_tensor("wv", [D + 1, VEXT], BF16, kind="ExternalInput").ap()
    wo = nc.dram_tensor("wo", [D + 1, D], F32R, kind="ExternalInput").ap()
    maskT = nc.dram_tensor("maskT", [TK, TQ], BF16, kind="ExternalInput").ap()
    qresT = nc.dram_tensor("qresT", [D, TQ], F32, kind="ExternalInput").ap()
    gam = nc.dram_tensor("gam", [1, D], F32, kind="ExternalInput").ap()
    bet = nc.dram_tensor("bet", [1, D], F32, kind="ExternalInput").ap()
    out = nc.dram_tensor("out", [TQ, D], F32, kind="ExternalOutput").ap()

    with TileContext(nc) as tc:
        import contextlib
        with contextlib.ExitStack() as ctx:
            pers = ctx.enter_context(tc.tile_pool(name="pers", bufs=1))
            dram = ctx.enter_context(tc.tile_pool(name="spill", bufs=1, space="DRAM"))

            # qhT_z: per head-parity zero-padded copies for K=128 QK matmuls
            qhT_z = pers.tile([128, NFT, 2, TQ], BF16)   # 32 KB/part

            attn_d = dram.tile([128, NFT, TQ], F32R)     # unnormalized attn out
            rr_d = dram.tile([H, TQ], F32)

            with tc.tile_pool(name="kvres", bufs=1) as kvres:
                vh_sb = kvres.tile([128, NKT, VEXT], BF16)   # 32.5 KB/part

                # ============ Phase P0: V projection + Q projection ========
                with tc.tile_pool(name="p0", bufs=1) as p0, \
                     tc.tile_pool(name="p0w", bufs=1) as p0w, \
                     tc.tile_pool(name="p0ps", bufs=2, space="PSUM") as p0ps:

                    # ---- V projection (bf16) -> vh_sb resident ----
                    wv_m = p0w.tile([128, NFT, VEXT], BF16, tag="w_m")
                    wv_b = p0w.tile([1, VEXT], BF16, tag="w_b")
                    nc.sync.dma_start(
                        out=wv_m, in_=wv[0:D, :].rearrange("(k p) f -> p k f", p=128))
                    nc.sync.dma_start(out=wv_b, in_=wv[D:D + 1, :])
                    vT_b = p0.tile([1, TK], BF16, tag="xv_b")
                    nc.sync.dma_start(out=vT_b, in_=vT[D:D + 1, :])
                    nchunks = [(0, 512), (512, 1024), (1024, VEXT)]
                    for half in range(2):
                        hs = slice(half * 1024, (half + 1) * 1024)
                        vT_m = p0.tile([128, NFT, 1024], BF16, tag="xv_m")
                        nc.sync.dma_start(
                            out=vT_m, in_=vT[0:D, hs].rearrange("(k p) t -> p k t", p=128))
                        for tl in range(8):
                            ti = half * 8 + tl
                            ps = p0ps.tile([128, 1536], F32, tag="pp")  # 3 banks
                            for (c0, c1) in nchunks:
                                for ki in range(NFT):
                                    nc.tensor.matmul(
                                        ps[:, c0:c1],
                                        vT_m[:, ki, tl * 128:(tl + 1) * 128],
                                        wv_m[:, ki, c0:c1],
                                        start=(ki == 0), stop=False)
                                nc.tensor.matmul(
                                    ps[:, c0:c1], vT_b[0:1, ti * 128:ti * 128 + 128],
                                    wv_b[0:1, c0:c1], start=False, stop=True)
                            if ti % 2 == 0:
                                nc.scalar.copy(vh_sb[:, ti, :], ps[:, 0:VEXT])
                            else:
                                nc.vector.tensor_copy(vh_sb[:, ti, :], ps[:, 0:VEXT])

                    # ---- Q projection (f32r) -> qhT_z resident ----
                    wq_m = p0w.tile([128, NFT, D], F32R, tag="wq_m")
                    wq_b = p0w.tile([1, D], F32R, tag="wq_b")
                    nc.sync.dma_start(
                        out=wq_m, in_=wq[0:D, :].rearrange("(k p) f -> p k f", p=128))
                    nc.sync.dma_start(out=wq_b, in_=wq[D:D + 1, :])
                    qT_m = p0.tile([128, NFT, TQ], F32R, tag="xq_m")
                    qT_b = p0.tile([1, TQ], F32R, tag="xq_b")
                    nc.sync.dma_start(
                        out=qT_m, in_=qT[0:D, :].rearrange("(k p) t -> p k t", p=128))
                    nc.sync.dma_start(out=qT_b, in_=qT[D:D + 1, :])
                    nc.vector.memset(qhT_z, 0.0)
                    for fi in range(NFT):
                        ps = p0ps.tile([128, 1536], F32, tag="pp")
                        for c in range(TQ // 512):
                            cs = slice(c * 512, (c + 1) * 512)
                            for ki in range(NFT):
                                nc.tensor.matmul(
                                    ps[:, cs], wq_m[:, ki, fi * 128:(fi + 1) * 128],
                                    qT_m[:, ki, cs], start=(ki == 0), stop=False)
                            nc.tensor.matmul(
                                ps[:, cs], wq_b[0:1, fi * 128:(fi + 1) * 128],
                                qT_b[0:1, cs], start=False, stop=True)
                        nc.scalar.copy(qhT_z[0:64, fi, 0, :], ps[0:64, 0:TQ])
                        nc.vector.tensor_copy(qhT_z[64:128, fi, 1, :], ps[64:128, 0:TQ])

                # ========== Phase A: attention (k-proj interleaved) ==========
                with tc.tile_pool(name="amask", bufs=1) as amask, \
                     tc.tile_pool(name="akh", bufs=2) as akh, \
                     tc.tile_pool(name="akp", bufs=1) as akp, \
                     tc.tile_pool(name="akq", bufs=2) as akq, \
                     tc.tile_pool(name="ap", bufs=5) as app, \
                     tc.tile_pool(name="aev", bufs=2) as aev, \
                     tc.tile_pool(name="aqk", bufs=2, space="PSUM") as aqk, \
                     tc.tile_pool(name="apv", bufs=1, space="PSUM") as apv, \
                     tc.tile_pool(name="akps", bufs=2, space="PSUM") as akps:

                    mk = amask.tile([128, NKT, TQ], BF16)
                    nc.sync.dma_start(
                        out=mk, in_=maskT.rearrange("(t p) q -> p t q", p=128))
                    wk_m = akp.tile([128, NFT, D], BF16)
                    wk_b = akp.tile([1, D], BF16)
                    nc.sync.dma_start(
                        out=wk_m, in_=wk[0:D, :].rearrange("(k p) f -> p k f", p=128))
                    nc.sync.dma_start(out=wk_b, in_=wk[D:D + 1, :])
                    kT_b = akp.tile([1, TK], BF16)
                    nc.sync.dma_start(out=kT_b, in_=kT[D:D + 1, :])

                    def emit_kproj(fi, khp):
                        # khp[:, :] = (k @ Wk + bk).T rows fi*128..fi*128+127
                        for qtr in range(4):
                            qs = slice(qtr * 512, (qtr + 1) * 512)
                            kT_q = akq.tile([128, NFT, 512], BF16, tag="ktq",
                                            name="ktq")
                            nc.sync.dma_start(
                                out=kT_q,
                                in_=kT[0:D, qs].rearrange("(k p) t -> p k t", p=128))
                            ps = akps.tile([128, 512], F32, tag="kp", name="kp")
                            for ki in range(NFT):
                                nc.tensor.matmul(
                                    ps, wk_m[:, ki, fi * 128:(fi + 1) * 128],
                                    kT_q[:, ki, :], start=(ki == 0), stop=False)
                            nc.tensor.matmul(
                                ps, wk_b[0:1, fi * 128:(fi + 1) * 128],
                                kT_b[0:1, qs], start=False, stop=True)
                            if qtr % 2 == 0:
                                nc.scalar.copy(khp[:, qs], ps)
                            else:
                                nc.vector.tensor_copy(khp[:, qs], ps)

                    work_q = []   # (pm, vsl, t, pv, j, hh, rs_p)

                    def emit_evac(pv, j, hh, rs_p):
                        h = 2 * j + hh
                        sh = aev.tile([65, TQ], F32R, tag="sh", name="sh")
                        if h % 2 == 0:
                            nc.scalar.copy(sh, pv[0:65, :])
                        else:
                            nc.vector.tensor_copy(sh, pv[0:65, :])
                        nc.sync.dma_start(
                            out=attn_d[64 * hh:64 * hh + 64, j, :], in_=sh[0:64, :])
                        nc.sync.dma_start(out=rs_p[hh:hh + 1, :],
                                          in_=sh[64:65, :].bitcast(F32))
                        if hh == 1:
                            # pair complete: reciprocal + stage for phase C
                            # (hidden under the ACT-bound attention stream)
                            rr_p = aev.tile([2, TQ], F32, tag="rrp", name="rr_p")
                            nc.vector.reciprocal(rr_p, rs_p)
                            nc.sync.dma_start(out=rr_d[2 * j:2 * j + 2, :],
                                              in_=rr_p)

                    def emit_pv():
                        pm, vsl, tp, pv, j, hh, rs_p = work_q.pop(0)
                        for c in range(TQ // 512):
                            cs = slice(c * 512, (c + 1) * 512)
                            nc.tensor.matmul(pv[:, cs], vh_sb[:, tp, vsl],
                                             pm[:, cs], start=(tp == 0),
                                             stop=(tp == NKT - 1))
                        if tp == NKT - 1:
                            emit_evac(pv, j, hh, rs_p)

                    for j in range(NPAIR):
                        khp = akh.tile([128, TK], BF16, tag="khp", name="khp")
                        rs_p = aev.tile([2, TQ], F32, tag="rsp", name="rs_p")
                        emit_kproj(j, khp)
                        for hh in range(2):
                            h = 2 * j + hh
                            pv = apv.tile([65, TQ], F32, tag="pv", name="pv")
                            vsl = slice(h * 65, h * 65 + 65)
                            for t in range(NKT):
                                tsl = slice(t * 128, (t + 1) * 128)
                                qk = aqk.tile([128, TQ], F32, tag="qk", name="qk")
                                for c in range(TQ // 512):
                                    cs = slice(c * 512, (c + 1) * 512)
                                    nc.tensor.matmul(qk[:, cs], khp[:, tsl],
                                                     qhT_z[:, j, hh, cs],
                                                     start=True, stop=True)
                                pe = app.tile([128, TQ], BF16, tag="pe", name="pe")
                                nc.scalar.activation(pe, qk, AF.Exp)
                                pm = app.tile([128, TQ], BF16, tag="pm", name="pm")
                                if t % 3 == 2:
                                    nc.gpsimd.tensor_mul(pm, pe, mk[:, t, :])
                                else:
                                    nc.vector.tensor_mul(pm, pe, mk[:, t, :])
                                work_q.append((pm, vsl, t, pv, j, hh, rs_p))
                                if len(work_q) > 3:
                                    emit_pv()
                    while work_q:
                        emit_pv()


            # ============ Phase C: normalize + out-proj + residual + LN ====
            with tc.tile_pool(name="cx", bufs=1) as cx, \
                 tc.tile_pool(name="cps", bufs=2, space="PSUM") as cps:

                attn_n = cx.tile([128, NFT, TQ], F32R)
                xT = cx.tile([128, NFT, TQ], F32)

                with tc.tile_pool(name="c0", bufs=1) as c0, \
                     tc.tile_pool(name="cq", bufs=2) as cq:
                    for ki in range(NFT):
                        at_t = cq.tile([128, TQ], F32R, tag="att", name="att")
                        nc.sync.dma_start(out=at_t, in_=attn_d[:, ki, :])
                        rrb = cq.tile([128, TQ], F32, tag="rrb", name="rrb")
                        nc.sync.dma_start(
                            out=rrb[0:64, :],
                            in_=rr_d[2 * ki:2 * ki + 1, :].broadcast_to((64, TQ)))
                        nc.sync.dma_start(
                            out=rrb[64:128, :],
                            in_=rr_d[2 * ki + 1:2 * ki + 2, :].broadcast_to((64, TQ)))
                        nc.vector.tensor_mul(attn_n[:, ki, :], at_t, rrb)

                    wo_m = c0.tile([128, NFT, D], F32R)
                    wo_b = c0.tile([1, D], F32R)
                    nc.sync.dma_start(
                        out=wo_m, in_=wo[0:D, :].rearrange("(k p) f -> p k f", p=128))
                    nc.sync.dma_start(out=wo_b, in_=wo[D:D + 1, :])
                    ones_f = c0.tile([1, TQ], F32)
                    nc.vector.memset(ones_f, 1.0)
                    ones_r = c0.tile([1, TQ], F32R)
                    nc.vector.tensor_scalar_mul(ones_r, ones_f, 1.0)

                    for f2 in range(NFT):
                        ps = cps.tile([128, TQ], F32, tag="pc")
                        f2s = slice(f2 * 128, (f2 + 1) * 128)
                        for c in range(TQ // 512):
                            cs_ = slice(c * 512, (c + 1) * 512)
                            for ki in range(NFT):
                                nc.tensor.matmul(ps[:, cs_], wo_m[:, ki, f2s],
                                                 attn_n[:, ki, cs_],
                                                 start=(ki == 0), stop=False)
                            nc.tensor.matmul(ps[:, cs_], wo_b[0:1, f2s],
                                             ones_r[0:1, cs_],
                                             start=False, stop=True)
                        qres_t = cq.tile([128, TQ], F32, tag="qres")
                        nc.sync.dma_start(out=qres_t, in_=qresT[f2s, :])
                        nc.vector.tensor_add(xT[:, f2, :], ps, qres_t)

                with tc.tile_pool(name="c1", bufs=1) as c1, \
                     tc.tile_pool(name="cl", bufs=2) as cl:
                    ident = c1.tile([128, 128], F32)
                    make_identity(nc, ident)
                    if not trivial_affine:
                        gam_r = c1.tile([1, D], F32)
                        bet_r = c1.tile([1, D], F32)
                        nc.sync.dma_start(out=gam_r, in_=gam)
                        nc.sync.dma_start(out=bet_r, in_=bet)
                        gam_b = c1.tile([128, D], F32)
                        bet_b = c1.tile([128, D], F32)
                        nc.gpsimd.partition_broadcast(gam_b, gam_r)
                        nc.gpsimd.partition_broadcast(bet_b, bet_r)
                    eps_t = c1.tile([128, 1], F32)
                    nc.vector.memset(eps_t, 1e-5)

                    for ti in range(NFT):
                        tis = slice(ti * 128, (ti + 1) * 128)
                        psx = cps.tile([128, D], F32, tag="pc")
                        for f2 in range(NFT):
                            nc.tensor.transpose(psx[:, f2 * 128:(f2 + 1) * 128],
                                                xT[:, f2, tis], ident)
                        stats = cl.tile([128, 2, 6], F32, tag="stats")
                        nc.vector.bn_stats(stats[:, 0, :], psx[:, 0:512])
                        nc.vector.bn_stats(stats[:, 1, :], psx[:, 512:1024])
                        mv = cl.tile([128, 2], F32, tag="mv")
                        nc.vector.bn_aggr(mv, stats)
                        sq = cl.tile([128, 1], F32, tag="sq")
                        nc.scalar.activation(sq, mv[:, 1:2], AF.Sqrt, bias=eps_t)
                        rstd = cl.tile([128, 1], F32, tag="rstd")
                        nc.vector.reciprocal(rstd, sq)
                        xo = cl.tile([128, D], F32, tag="xo")
                        nc.vector.tensor_scalar(xo, psx, mv[:, 0:1], rstd,
                                                op0=mybir.AluOpType.subtract,
                                                op1=mybir.AluOpType.mult)
                        if not trivial_affine:
                            nc.vector.tensor_mul(xo, xo, gam_b)
                            nc.vector.tensor_add(xo, xo, bet_b)
                        nc.sync.dma_start(out=out[tis, :], in_=xo)
    return nc


def _prep_core_inputs(inputs, b, qh):
    """Build the per-core input map (host-side layout prep only)."""
    import ml_dtypes
    q = np.asarray(inputs["q"], np.float32)
    k = np.asarray(inputs["k"], np.float32)
    v = np.asarray(inputs["v"], np.float32)
    mask = np.asarray(inputs["attn_mask"])
    Wq, bq = np.asarray(inputs["Wq"], np.float32), np.asarray(inputs["bq"], np.float32)
    Wk, bk = np.asarray(inputs["Wk"], np.float32), np.asarray(inputs["bk"], np.float32)
    Wv, bv = np.asarray(inputs["Wv"], np.float32), np.asarray(inputs["bv"], np.float32)
    Wo, bo = np.asarray(inputs["Wo"], np.float32), np.asarray(inputs["bo"], np.float32)
    gamma, beta = np.asarray(inputs["gamma"], np.float32), np.asarray(inputs["beta"], np.float32)

    qs = slice(qh * TQ, (qh + 1) * TQ)
    qb = q[b, qs, :]                       # [TQ, D]

    def ext_T(x_t):  # [D, N] -> [D+1, N] with ones row
        return np.concatenate([x_t, np.ones((1, x_t.shape[1]), np.float32)], axis=0)

    def ext_W(W, bias):  # [D, N] -> [D+1, N] with bias row
        return np.concatenate([W, bias[None, :]], axis=0)

    # Wv extended with per-head ones column: col h*65+64 gets bias 1, weights 0
    Wv_ext = np.zeros((D, VEXT), np.float32)
    bv_ext = np.zeros((VEXT,), np.float32)
    for h in range(H):
        Wv_ext[:, h * 65:h * 65 + 64] = Wv[:, h * 64:(h + 1) * 64]
        bv_ext[h * 65:h * 65 + 64] = bv[h * 64:(h + 1) * 64]
        bv_ext[h * 65 + 64] = 1.0

    bf = ml_dtypes.bfloat16
    return {
        "qT": _round_f32r(ext_T(qb.T.copy())),
        "kT": ext_T(k[b].T.copy()).astype(bf),
        "vT": ext_T(v[b].T.copy()).astype(bf),
        "wq": _round_f32r(ext_W(Wq, bq)),
        "wk": ext_W(Wk, bk).astype(bf),
        "wv": ext_W(Wv_ext, bv_ext).astype(bf),
        "wo": _round_f32r(ext_W(Wo, bo)),
        "maskT": np.ascontiguousarray(mask[b, qs, :].T).astype(bf),
        "qresT": np.ascontiguousarray(qb.T),
        "gam": gamma[None, :].copy(),
        "bet": beta[None, :].copy(),
    }


def kernel(**inputs) -> np.ndarray:
    global _LAST_RESULTS
    trivial_affine = (np.all(np.asarray(inputs["gamma"]) == 1.0)
                      and np.all(np.asarray(inputs["beta"]) == 0.0))
    nc = bacc.Bacc("TRN2", debug=False, num_devices=NCORES)
    build_program(nc, trivial_affine=trivial_affine)
    nc.finalize()

    ncores_run = int(os.environ.get("KERNEL_NCORES", str(NCORES)))
    in_maps = [_prep_core_inputs(inputs, c // 2, c % 2) for c in range(NCORES)]
    trace = bool(int(os.environ.get("KERNEL_TRACE", "0")))
    res = run_bass_kernel_spmd(nc, in_maps[:ncores_run],
                               core_ids=list(range(ncores_run)), trace=trace)
    _LAST_RESULTS = {"exec_time_ns": res.exec_time_ns,
                     "profile_json": res.profile_json,
                     "res": res}

    out = np.empty((B, T, D), np.float32)
    for c in range(NCORES):
        b, qh = c // 2, c % 2
        out[b, qh * TQ:(qh + 1) * TQ, :] = res.results[c % ncores_run]["out"]
    return out


# revision 41
# speedup vs baseline: 1.0504x; 1.0504x over previous
"""Trainium2 Bass kernel for nn_MultiHeadAttention_76587856823057.

Sharding: (batch, query-half) -> 8 cores, zero collectives.
Per core: b fixed, queries TQ=1024 (half of T), all H=16 heads, all TK=2048 keys.

Math notes vs reference:
 - softmax is shift-invariant; the reference's *global* max subtract cancels in
   the normalization (the +1e-15 in the denominator is ~1e-12 relative), and
   scores are bounded (|s| < ~30) so exp() cannot overflow in fp32. We skip the
   max pass entirely.
 - exp(s*m)*m == exp(s)*m for m in {0,1}: one mask multiply only.
 - row sums come free from the PV matmul via a ones-column per head (M=65).
 - Q projection runs in float32r (fp32 with 11-bit-mantissa inputs, fp32
   accumulate, full PE speed); K/V projections, scores and probabilities in
   bf16 (fast weight loads, no fp32 two-pass weight path).
 - QK contracts K=128 against parity-zero-padded qhT_z so every matmul stays
   in plain 128x128 PE mode (no tile-mode switching / drains).
 - K-projection is interleaved into the (ACT-bound) attention phase, one
   feature-tile block per pair, directly producing each pair's khp SBUF tile.
 - biases fold into the matmuls via an appended ones-row (K=1025).

Self-contained: hardcodes all shapes; no sibling imports.
"""

import os
import numpy as np

import concourse.bass as bass
from concourse import bacc
import concourse.mybir as mybir
from concourse.tile import TileContext
from concourse.bass_utils import run_bass_kernel_spmd
from concourse.masks import make_identity

F32 = mybir.dt.float32
F32R = mybir.dt.float32r
BF16 = mybir.dt.bfloat16
AF = mybir.ActivationFunctionType

B, T, D, H, DK = 4, 2048, 1024, 16, 64
TQ = T // 2          # queries per core
TK = T               # keys per core
NCORES = 8
NPAIR = H // 2       # 8 head pairs
NFT = D // 128       # 8 feature tiles
NKT = TK // 128      # 16 key tiles
VEXT = H * (DK + 1)  # 1040: per-head [64 v-cols + ones col]

_LAST_RESULTS = {}


def _round_f32r(x: np.ndarray) -> np.ndarray:
    """Round fp32 to the PE's fp32r input format (11-bit mantissa)."""
    bits = np.ascontiguousarray(x, dtype=np.float32).view(np.uint32)
    out = ((bits.astype(np.uint64) + 0x800) & 0xFFFFF000).astype(np.uint32)
    return out.view(np.float32)


def build_program(nc: bass.Bass, trivial_affine: bool = False):
    # ---- per-core DRAM I/O ----
    qT = nc.dram_tensor("qT", [D + 1, TQ], F32R, kind="ExternalInput").ap()
    kT = nc.dram_tensor("kT", [D + 1, TK], BF16, kind="ExternalInput").ap()
    vT = nc.dram_tensor("vT", [D + 1, TK], BF16, kind="ExternalInput").ap()
    wq = nc.dram_tensor("wq", [D + 1, D], F32R, kind="ExternalInput").ap()
    wk = nc.dram_tensor("wk", [D + 1, D], BF16, kind="ExternalInput").ap()
    wv = nc.dram_tensor("wv", [D + 1, VEXT], BF16, kind="ExternalInput").ap()
    wo = nc.dram_tensor("wo", [D + 1, D], F32R, kind="ExternalInput").ap()
    maskT = nc.dram_tensor("maskT", [TK, TQ], BF16, kind="ExternalInput").ap()
    qresT = nc.dram_tensor("qresT", [D, TQ], F32, kind="ExternalInput").ap()
    gam = nc.dram_tensor("gam", [1, D], F32, kind="ExternalInput").ap()
    bet = nc.dram_tensor("bet", [1, D], F32, kind="ExternalInput").ap()
    out = nc.dram_tensor("out", [TQ, D], F32, kind="ExternalOutput").ap()

    with TileContext(nc) as tc:
        import contextlib
        with contextlib.ExitStack() as ctx:
            pers = ctx.enter_context(tc.tile_pool(name="pers", bufs=1))
            dram = ctx.enter_context(tc.tile_pool(name="spill", bufs=1, space="DRAM"))

            # qhT_z: per head-parity zero-padded copies for K=128 QK matmuls
            qhT_z = pers.tile([128, NFT, 2, TQ], BF16)   # 32 KB/part

            rs_all = pers.tile([H, TQ], F32)             # row sums per head
            rr_all = pers.tile([H, TQ], F32)             # reciprocals

            attn_d = dram.tile([128, NFT, TQ], F32R)     # unnormalized attn out
            rr_d = dram.tile([H, TQ], F32)

            with tc.tile_pool(name="kvres", bufs=1) as kvres:
                vh_sb = kvres.tile([128, NKT, VEXT], BF16)   # 32.5 KB/part

                # ============ Phase P0: V projection + Q projection ========
                with tc.tile_pool(name="p0", bufs=1) as p0, \
                     tc.tile_pool(name="p0w", bufs=1) as p0w, \
                     tc.tile_pool(name="p0ps", bufs=2, space="PSUM") as p0ps:

                    # ---- V projection (bf16) -> vh_sb resident ----
                    wv_m = p0w.tile([128, NFT, VEXT], BF16, tag="w_m")
                    wv_b = p0w.tile([1, VEXT], BF16, tag="w_b")
                    nc.sync.dma_start(
                        out=wv_m, in_=wv[0:D, :].rearrange("(k p) f -> p k f", p=128))
                    nc.sync.dma_start(out=wv_b, in_=wv[D:D + 1, :])
                    vT_b = p0.tile([1, TK], BF16, tag="xv_b")
                    nc.sync.dma_start(out=vT_b, in_=vT[D:D + 1, :])
                    nchunks = [(0, 512), (512, 1024), (1024, VEXT)]
                    for half in range(2):
                        hs = slice(half * 1024, (half + 1) * 1024)
                        vT_m = p0.tile([128, NFT, 1024], BF16, tag="xv_m")
                        nc.sync.dma_start(
                            out=vT_m, in_=vT[0:D, hs].rearrange("(k p) t -> p k t", p=128))
                        for tl in range(8):
                            ti = half * 8 + tl
                            ps = p0ps.tile([128, 1536], F32, tag="pp")  # 3 banks
                            for (c0, c1) in nchunks:
                                for ki in range(NFT):
                                    nc.tensor.matmul(
                                        ps[:, c0:c1],
                                        vT_m[:, ki, tl * 128:(tl + 1) * 128],
                                        wv_m[:, ki, c0:c1],
                                        start=(ki == 0), stop=False)
                                nc.tensor.matmul(
                                    ps[:, c0:c1], vT_b[0:1, ti * 128:ti * 128 + 128],
                                    wv_b[0:1, c0:c1], start=False, stop=True)
                            if ti % 2 == 0:
                                nc.scalar.copy(vh_sb[:, ti, :], ps[:, 0:VEXT])
                            else:
                                nc.vector.tensor_copy(vh_sb[:, ti, :], ps[:, 0:VEXT])

                    # ---- Q projection (f32r) -> qhT_z resident ----
                    wq_m = p0w.tile([128, NFT, D], F32R, tag="wq_m")
                    wq_b = p0w.tile([1, D], F32R, tag="wq_b")
                    nc.sync.dma_start(
                        out=wq_m, in_=wq[0:D, :].rearrange("(k p) f -> p k f", p=128))
                    nc.sync.dma_start(out=wq_b, in_=wq[D:D + 1, :])
                    qT_m = p0.tile([128, NFT, TQ], F32R, tag="xq_m")
                    qT_b = p0.tile([1, TQ], F32R, tag="xq_b")
                    nc.sync.dma_start(
                        out=qT_m, in_=qT[0:D, :].rearrange("(k p) t -> p k t", p=128))
                    nc.sync.dma_start(out=qT_b, in_=qT[D:D + 1, :])
                    nc.vector.memset(qhT_z, 0.0)
                    for fi in range(NFT):
                        ps = p0ps.tile([128, 1536], F32, tag="pp")
                        for c in range(TQ // 512):
                            cs = slice(c * 512, (c + 1) * 512)
                            for ki in range(NFT):
                                nc.tensor.matmul(
                                    ps[:, cs], wq_m[:, ki, fi * 128:(fi + 1) * 128],
                                    qT_m[:, ki, cs], start=(ki == 0), stop=False)
                            nc.tensor.matmul(
                                ps[:, cs], wq_b[0:1, fi * 128:(fi + 1) * 128],
                                qT_b[0:1, cs], start=False, stop=True)
                        nc.scalar.copy(qhT_z[0:64, fi, 0, :], ps[0:64, 0:TQ])
                        nc.vector.tensor_copy(qhT_z[64:128, fi, 1, :], ps[64:128, 0:TQ])

                # ========== Phase A: attention (k-proj interleaved) ==========
                with tc.tile_pool(name="amask", bufs=1) as amask, \
                     tc.tile_pool(name="akh", bufs=2) as akh, \
                     tc.tile_pool(name="akp", bufs=1) as akp, \
                     tc.tile_pool(name="akq", bufs=2) as akq, \
                     tc.tile_pool(name="ap", bufs=5) as app, \
                     tc.tile_pool(name="aev", bufs=2) as aev, \
                     tc.tile_pool(name="aqk", bufs=2, space="PSUM") as aqk, \
                     tc.tile_pool(name="apv", bufs=1, space="PSUM") as apv, \
                     tc.tile_pool(name="akps", bufs=2, space="PSUM") as akps:

                    mk = amask.tile([128, NKT, TQ], BF16)
                    nc.sync.dma_start(
                        out=mk, in_=maskT.rearrange("(t p) q -> p t q", p=128))
                    wk_m = akp.tile([128, NFT, D], BF16)
                    wk_b = akp.tile([1, D], BF16)
                    nc.sync.dma_start(
                        out=wk_m, in_=wk[0:D, :].rearrange("(k p) f -> p k f", p=128))
                    nc.sync.dma_start(out=wk_b, in_=wk[D:D + 1, :])
                    kT_b = akp.tile([1, TK], BF16)
                    nc.sync.dma_start(out=kT_b, in_=kT[D:D + 1, :])

                    def emit_kproj(fi, khp):
                        # khp[:, :] = (k @ Wk + bk).T rows fi*128..fi*128+127
                        for qtr in range(4):
                            qs = slice(qtr * 512, (qtr + 1) * 512)
                            kT_q = akq.tile([128, NFT, 512], BF16, tag="ktq",
                                            name="ktq")
                            nc.sync.dma_start(
                                out=kT_q,
                                in_=kT[0:D, qs].rearrange("(k p) t -> p k t", p=128))
                            ps = akps.tile([128, 512], F32, tag="kp", name="kp")
                            for ki in range(NFT):
                                nc.tensor.matmul(
                                    ps, wk_m[:, ki, fi * 128:(fi + 1) * 128],
                                    kT_q[:, ki, :], start=(ki == 0), stop=False)
                            nc.tensor.matmul(
                                ps, wk_b[0:1, fi * 128:(fi + 1) * 128],
                                kT_b[0:1, qs], start=False, stop=True)
                            if qtr % 2 == 0:
                                nc.scalar.copy(khp[:, qs], ps)
                            else:
                                nc.vector.tensor_copy(khp[:, qs], ps)

                    work_q = []   # (pm, vsl, t, pv, j, hh, rs_p)

                    def emit_evac(pv, j, hh, rs_p):
                        h = 2 * j + hh
                        sh = aev.tile([65, TQ], F32R, tag="sh", name="sh")
                        if h % 2 == 0:
                            nc.scalar.copy(sh, pv[0:65, :])
                        else:
                            nc.vector.tensor_copy(sh, pv[0:65, :])
                        nc.sync.dma_start(
                            out=attn_d[64 * hh:64 * hh + 64, j, :], in_=sh[0:64, :])
                        nc.sync.dma_start(out=rs_all[h:h + 1, :],
                                          in_=sh[64:65, :].bitcast(F32))

                    def emit_pv():
                        pm, vsl, tp, pv, j, hh, rs_p = work_q.pop(0)
                        for c in range(TQ // 512):
                            cs = slice(c * 512, (c + 1) * 512)
                            nc.tensor.matmul(pv[:, cs], vh_sb[:, tp, vsl],
                                             pm[:, cs], start=(tp == 0),
                                             stop=(tp == NKT - 1))
                        if tp == NKT - 1:
                            emit_evac(pv, j, hh, rs_p)

                    for j in range(NPAIR):
                        khp = akh.tile([128, TK], BF16, tag="khp", name="khp")
                        rs_p = None
                        emit_kproj(j, khp)
                        for hh in range(2):
                            h = 2 * j + hh
                            pv = apv.tile([65, TQ], F32, tag="pv", name="pv")
                            vsl = slice(h * 65, h * 65 + 65)
                            for t in range(NKT):
                                tsl = slice(t * 128, (t + 1) * 128)
                                qk = aqk.tile([128, TQ], F32, tag="qk", name="qk")
                                for c in range(TQ // 512):
                                    cs = slice(c * 512, (c + 1) * 512)
                                    nc.tensor.matmul(qk[:, cs], khp[:, tsl],
                                                     qhT_z[:, j, hh, cs],
                                                     start=True, stop=True)
                                pe = app.tile([128, TQ], BF16, tag="pe", name="pe")
                                nc.scalar.activation(pe, qk, AF.Exp)
                                pm = app.tile([128, TQ], BF16, tag="pm", name="pm")
                                if t % 3 == 2:
                                    nc.gpsimd.tensor_mul(pm, pe, mk[:, t, :])
                                else:
                                    nc.vector.tensor_mul(pm, pe, mk[:, t, :])
                                work_q.append((pm, vsl, t, pv, j, hh, rs_p))
                                if len(work_q) > 3:
                                    emit_pv()
                    while work_q:
                        emit_pv()

                    nc.vector.reciprocal(rr_all, rs_all)
                    nc.sync.dma_start(out=rr_d, in_=rr_all)


            # ============ Phase C: normalize + out-proj + residual + LN ====
            with tc.tile_pool(name="cx", bufs=1) as cx, \
                 tc.tile_pool(name="cps", bufs=2, space="PSUM") as cps:

                attn_n = cx.tile([128, NFT, TQ], F32R)
                xT = cx.tile([128, NFT, TQ], F32)

                with tc.tile_pool(name="c0", bufs=1) as c0, \
                     tc.tile_pool(name="cq", bufs=2) as cq:
                    for ki in range(NFT):
                        at_t = cq.tile([128, TQ], F32R, tag="att", name="att")
                        nc.sync.dma_start(out=at_t, in_=attn_d[:, ki, :])
                        rrb = cq.tile([128, TQ], F32, tag="rrb", name="rrb")
                        nc.sync.dma_start(
                            out=rrb[0:64, :],
                            in_=rr_d[2 * ki:2 * ki + 1, :].broadcast_to((64, TQ)))
                        nc.sync.dma_start(
                            out=rrb[64:128, :],
                            in_=rr_d[2 * ki + 1:2 * ki + 2, :].broadcast_to((64, TQ)))
                        nc.vector.tensor_mul(attn_n[:, ki, :], at_t, rrb)

                    wo_m = c0.tile([128, NFT, D], F32R)
                    wo_b = c0.tile([1, D], F32R)
                    nc.sync.dma_start(
                        out=wo_m, in_=wo[0:D, :].rearrange("(k p) f -> p k f", p=128))
                    nc.sync.dma_start(out=wo_b, in_=wo[D:D + 1, :])
                    ones_f = c0.tile([1, TQ], F32)
                    nc.vector.memset(ones_f, 1.0)
                    ones_r = c0.tile([1, TQ], F32R)
                    nc.vector.tensor_scalar_mul(ones_r, ones_f, 1.0)

                    for f2 in range(NFT):
                        ps = cps.tile([128, TQ], F32, tag="pc")
                        f2s = slice(f2 * 128, (f2 + 1) * 128)
                        for c in range(TQ // 512):
                            cs_ = slice(c * 512, (c + 1) * 512)
                            for ki in range(NFT):
                                nc.tensor.matmul(ps[:, cs_], wo_m[:, ki, f2s],
                                                 attn_n[:, ki, cs_],
                                                 start=(ki == 0), stop=False)
                            nc.tensor.matmul(ps[:, cs_], wo_b[0:1, f2s],
                                             ones_r[0:1, cs_],
                                             start=False, stop=True)
                        qres_t = cq.tile([128, TQ], F32, tag="qres")
                        nc.sync.dma_start(out=qres_t, in_=qresT[f2s, :])
                        nc.vector.tensor_add(xT[:, f2, :], ps, qres_t)

                with tc.tile_pool(name="c1", bufs=1) as c1, \
                     tc.tile_pool(name="cl", bufs=2) as cl:
                    ident = c1.tile([128, 128], F32)
                    make_identity(nc, ident)
                    if not trivial_affine:
                        gam_r = c1.tile([1, D], F32)
                        bet_r = c1.tile([1, D], F32)
                        nc.sync.dma_start(out=gam_r, in_=gam)
                        nc.sync.dma_start(out=bet_r, in_=bet)
                        gam_b = c1.tile([128, D], F32)
                        bet_b = c1.tile([128, D], F32)
                        nc.gpsimd.partition_broadcast(gam_b, gam_r)
                        nc.gpsimd.partition_broadcast(bet_b, bet_r)
                    eps_t = c1.tile([128, 1], F32)
                    nc.vector.memset(eps_t, 1e-5)

                    for ti in range(NFT):
                        tis = slice(ti * 128, (ti + 1) * 128)
                        psx = cps.tile([128, D], F32, tag="pc")
                        for f2 in range(NFT):
                            nc.tensor.transpose(psx[:, f2 * 128:(f2 + 1) * 128],
                                                xT[:, f2, tis], ident)
                        stats = cl.tile([128, 2, 6], F32, tag="stats")
                        nc.vector.bn_stats(stats[:, 0, :], psx[:, 0:512])
                        nc.vector.bn_stats(stats[:, 1, :], psx[:, 512:1024])
                        mv = cl.tile([128, 2], F32, tag="mv")
                        nc.vector.bn_aggr(mv, stats)
                        sq = cl.tile([128, 1], F32, tag="sq")
                        nc.scalar.activation(sq, mv[:, 1:2], AF.Sqrt, bias=eps_t)
                        rstd = cl.tile([128, 1], F32, tag="rstd")
                        nc.vector.reciprocal(rstd, sq)
                        xo = cl.tile([128, D], F32, tag="xo")
                        nc.vector.tensor_scalar(xo, psx, mv[:, 0:1], rstd,
                                                op0=mybir.AluOpType.subtract,
                                                op1=mybir.AluOpType.mult)
                        if not trivial_affine:
                            nc.vector.tensor_mul(xo, xo, gam_b)
                            nc.vector.tensor_add(xo, xo, bet_b)
                        nc.sync.dma_start(out=out[tis, :], in_=xo)
    return nc


def _prep_core_inputs(inputs, b, qh):
    """Build the per-core input map (host-side layout prep only)."""
    import ml_dtypes
    q = np.asarray(inputs["q"], np.float32)
    k = np.asarray(inputs["k"], np.float32)
    v = np.asarray(inputs["v"], np.float32)
    mask = np.asarray(inputs["attn_mask"])
    Wq, bq = np.asarray(inputs["Wq"], np.float32), np.asarray(inputs["bq"], np.float32)
    Wk, bk = np.asarray(inputs["Wk"], np.float32), np.asarray(inputs["bk"], np.float32)
    Wv, bv = np.asarray(inputs["Wv"], np.float32), np.asarray(inputs["bv"], np.float32)
    Wo, bo = np.asarray(inputs["Wo"], np.float32), np.asarray(inputs["bo"], np.float32)
    gamma, beta = np.asarray(inputs["gamma"], np.float32), np.asarray(inputs["beta"], np.float32)

    qs = slice(qh * TQ, (qh + 1) * TQ)
    qb = q[b, qs, :]                       # [TQ, D]

    def ext_T(x_t):  # [D, N] -> [D+1, N] with ones row
        return np.concatenate([x_t, np.ones((1, x_t.shape[1]), np.float32)], axis=0)

    def ext_W(W, bias):  # [D, N] -> [D+1, N] with bias row
        return np.concatenate([W, bias[None, :]], axis=0)

    # Wv extended with per-head ones column: col h*65+64 gets bias 1, weights 0
    Wv_ext = np.zeros((D, VEXT), np.float32)
    bv_ext = np.zeros((VEXT,), np.float32)
    for h in range(H):
        Wv_ext[:, h * 65:h * 65 + 64] = Wv[:, h * 64:(h + 1) * 64]
        bv_ext[h * 65:h * 65 + 64] = bv[h * 64:(h + 1) * 64]
        bv_ext[h * 65 + 64] = 1.0

    bf = ml_dtypes.bfloat16
    return {
        "qT": _round_f32r(ext_T(qb.T.copy())),
        "kT": ext_T(k[b].T.copy()).astype(bf),
        "vT": ext_T(v[b].T.copy()).astype(bf),
        "wq": _round_f32r(ext_W(Wq, bq)),
        "wk": ext_W(Wk, bk).astype(bf),
        "wv": ext_W(Wv_ext, bv_ext).astype(bf),
        "wo": _round_f32r(ext_W(Wo, bo)),
        "maskT": np.ascontiguousarray(mask[b, qs, :].T).astype(bf),
        "qresT": np.ascontiguousarray(qb.T),
        "gam": gamma[None, :].copy(),
        "bet": beta[None, :].copy(),
    }


def kernel(**inputs) -> np.ndarray:
    global _LAST_RESULTS
    trivial_affine = (np.all(np.asarray(inputs["gamma"]) == 1.0)
                      and np.all(np.asarray(inputs["beta"]) == 0.0))
    nc = bacc.Bacc("TRN2", debug=False, num_devices=NCORES)
    build_program(nc, trivial_affine=trivial_affine)
    nc.finalize()

    ncores_run = int(os.environ.get("KERNEL_NCORES", str(NCORES)))
    in_maps = [_prep_core_inputs(inputs, c // 2, c % 2) for c in range(NCORES)]
    trace = bool(int(os.environ.get("KERNEL_TRACE", "0")))
    res = run_bass_kernel_spmd(nc, in_maps[:ncores_run],
                               core_ids=list(range(ncores_run)), trace=trace)
    _LAST_RESULTS = {"exec_time_ns": res.exec_time_ns,
                     "profile_json": res.profile_json,
                     "res": res}

    out = np.empty((B, T, D), np.float32)
    for c in range(NCORES):
        b, qh = c // 2, c % 2
        out[b, qh * TQ:(qh + 1) * TQ, :] = res.results[c % ncores_run]["out"]
    return out
